# revision 22
# baseline (speedup 1.0000x reference)
"""AttentionalSplatting TRN2 kernel.

Sharding: data-parallel over T (16 timesteps) across 8 cores, 2 timesteps per
core. Weights are baked into the NEFF as constants (loaded to HBM once at
model-load; they never travel per call). Host does layout permutations,
dtype casts and int8 quantization only; all arithmetic runs on device.

Per-call traffic is the optimization target on this link (~40 MB/s H2D,
~25 MB/s D2H, ~80 ms RTT per op):
  - ONE packed int8 input tensor per core [T_PER_CORE, D, 1796]:
    columns [fpeT_i8 | tpeT_i8 | uttT-as-raw-fp16-bytes |
    tracks-as-raw-f32-bytes]. fpe/tpe rows are quantized per token row
    with NO shipped scale: QK-LayerNorm is scale-invariant per row, so
    the quantization scale cancels exactly on device. utt (V path) has
    no LN to absorb a scale and int8 was too lossy there, so it travels
    as fp16 raw bytes recovered with AP.bitcast. tracks need exact f32
    for the bias-cancellation trick and ride along the same way.
  - Output is int8 [T_PER_CORE, HW, 516]: 512 quantized values per row
    plus that row's f32 dequant scale bit-packed in the last 4 bytes.
    Host dequant is one fused np.multiply.
  - run_bass_via_pjrt rebuilds its jit wrapper per call, defeating jax's
    compile caches; two content-keyed memos below (neuronx_cc hook and
    backend_compile_and_load) recover the ~0.6 s/call that was spent
    re-verifying the identical BIR and re-shipping the identical
    executable. Data transfer + device execution still happen per call.

Per-timestep device pipeline (fp16 matmuls, fp32 softmax/LN statistics;
exp/V tiles are bf16 because fp16's 6e-8 floor underflows to a zero
softmax denominator for far-from-every-track queries — row-max scores
reach -40 on real inputs, and bf16's 1e-38 range absorbs that):
  Q = fpe @ WqT   (natural [q, dk] layout, PSUM)    -> LN stats -> apply -> fp16
  K = tpe @ WkT   likewise; V = utt @ WvT -> V-hat [k, 8, 65] with ones col
  Qln/Kln PE-transposed to [dk, q]; gamma_q*gamma_k/8 folded into K side.
  scoresT[k,q] per head = Kh^T.T @ Qh^T  (+ spatial bias via a rank-6 f32r
  matmul on appended position rows: -2*d2 = 4 tr.fp - 2|tr|^2 - 2|fp|^2)
  exp on ACT (no max subtraction needed: bias <= 0, |QK/8| small)
  U_h[q, 65] = expS^T.T @ Vhat_h  (col 64 = softmax denom) -> recip -> scale
  out = U @ WoT via PE transpose of U, per-row int8 quantization, DMA out.
"""

import hashlib
import os
from contextlib import ExitStack

import numpy as np

import concourse.bass as bass
import concourse.mybir as mybir
import concourse.tile as tile
from concourse import bacc, bass2jax, bass_utils
from concourse.masks import make_identity

# run_bass_via_pjrt rebuilds its jitted callable every call, so jax's
# compile caches miss and concourse's neuronx_cc hook re-runs the full
# BIR verify/optimise + DVE-table pipeline (~0.6 s of client CPU) on
# byte-identical input each call. The hook is a pure function of the
# serialized HLO (the NEFF repack is explicitly made deterministic via
# _reset_tarinfo/make_deterministic_neff_header), so memoize it by
# content hash — same role as the NEFF compile cache in the bench
# containers. Execution itself still runs on hardware every call.
_NEFF_MEMO = {}
_ORIG_NEURONX_CC_HOOK = bass2jax.neuronx_cc_hook


def _memo_neuronx_cc_hook(code, code_format, platform_version, file_prefix):
    try:
        key = (
            hashlib.sha256(bytes(code)).digest(),
            bytes(code_format),
            str(platform_version),
        )
    except Exception:
        return _ORIG_NEURONX_CC_HOOK(code, code_format, platform_version, file_prefix)
    hit = _NEFF_MEMO.get(key)
    if hit is None:
        hit = _ORIG_NEURONX_CC_HOOK(code, code_format, platform_version, file_prefix)
        _NEFF_MEMO[key] = hit
    return hit


bass2jax.neuronx_cc_hook = _memo_neuronx_cc_hook

# Same story one level up: jax's own executable cache keys miss for the
# per-call-rebuilt jit wrapper, so every call re-runs backend.compile
# (client-side NEFF wrap + shipping the executable to the axon terminal,
# ~0.6 s). The compile is deterministic in (module bytecode, compile
# options, devices) — memoize the loaded executable on that key, exactly
# the hit-path jax's _cached_compilation intends. Fall through to the
# original on any surprise.
_EXE_MEMO = {}


def _install_compile_memo():
    import jax._src.compiler as _jc
    from jax._src.interpreters import mlir as _jmlir

    if getattr(_jc.backend_compile_and_load, "_kernel_memo", False):
        return
    _orig = _jc.backend_compile_and_load

    def _memo(backend, module, executable_devices, options, host_callbacks):
        try:
            if host_callbacks:
                return _orig(
                    backend, module, executable_devices, options, host_callbacks
                )
            key = (
                hashlib.sha256(_jmlir.module_to_bytecode(module)).digest(),
                options.SerializeAsString(),
                tuple(d.id for d in executable_devices),
            )
        except Exception:
            return _orig(backend, module, executable_devices, options, host_callbacks)
        hit = _EXE_MEMO.get(key)
        if hit is None:
            hit = _orig(backend, module, executable_devices, options, host_callbacks)
            _EXE_MEMO[key] = hit
        return hit

    _memo._kernel_memo = True
    _jc.backend_compile_and_load = _memo


_install_compile_memo()

F32 = mybir.dt.float32
F32R = mybir.dt.float32r
BF16 = mybir.dt.bfloat16
FP16 = mybir.dt.float16
I8 = mybir.dt.int8

T_PER_CORE = 2
N_CORES = 8
HW = 1024  # queries
M = 256    # tracks/keys
D = 512    # d_model = d_k
H = 8
HD = 64
EPS = 1e-6
ACT_W = HW + M + 2 * M + 4  # packed: [fpeT i8 | tpeT i8 | uttT fp16-bytes | trk]
UTT_C0 = HW + M             # utt fp16 region start (as int8 columns)
TRK_C0 = HW + 3 * M         # tracks f32 bytes region start
OUT_W = D + 4           # int8 values + bit-packed f32 row scale
QSCALE = 126.5          # int8 target amplitude (rounding can add 0.5)

LAST_RESULT = None


def _build_bass(consts, t_per_core=T_PER_CORE):
    nc = bacc.Bacc("TRN2", target_bir_lowering=False)

    actT = nc.dram_tensor("actT", [t_per_core, D, ACT_W], I8, kind="ExternalInput").ap()
    out = nc.dram_tensor("out", [t_per_core, HW, OUT_W], I8, kind="ExternalOutput").ap()

    # Weight-like tensors baked into the NEFF: loaded to HBM at model load,
    # zero per-call transfer cost.
    wqT = nc.inline_tensor(consts["wqT"], name="wqT").ap()
    wkT = nc.inline_tensor(consts["wkT"], name="wkT").ap()
    wvT = nc.inline_tensor(consts["wvT"], name="wvT").ap()
    woT = nc.inline_tensor(consts["woT"], name="woT").ap()
    gqk = nc.inline_tensor(consts["gqk"], name="gqk").ap()
    fpT = nc.inline_tensor(consts["fpT"], name="fpT").ap()

    with tile.TileContext(nc) as tc, ExitStack() as ctx:
        singles = ctx.enter_context(tc.tile_pool(name="singles", bufs=1))
        ins = ctx.enter_context(tc.tile_pool(name="ins", bufs=1))
        work = ctx.enter_context(tc.tile_pool(name="work", bufs=2))
        work1 = ctx.enter_context(tc.tile_pool(name="work1", bufs=1))
        small = ctx.enter_context(tc.tile_pool(name="small", bufs=2))
        exps = ctx.enter_context(tc.tile_pool(name="exps", bufs=16))
        outs = ctx.enter_context(tc.tile_pool(name="outs", bufs=2))
        pA = ctx.enter_context(tc.tile_pool(name="pA", bufs=2, space="PSUM"))
        pS = ctx.enter_context(tc.tile_pool(name="pS", bufs=2, space="PSUM"))
        dscr = ctx.enter_context(tc.tile_pool(name="dscr", bufs=2, space="DRAM"))

        # ---- one-time constants ----
        ident = singles.tile([128, 128], FP16)
        make_identity(nc, ident)

        w_sb = {}
        for name, ap in (("wq", wqT), ("wk", wkT), ("wv", wvT), ("wo", woT)):
            wt = singles.tile([128, 4, D], FP16, tag=name)
            nc.gpsimd.dma_start(out=wt, in_=ap.rearrange("(c p) n -> p c n", p=128))
            w_sb[name] = wt

        # ext rows (rank-6 bias matmul):
        #   lhsT_ext [6, M]  = [tr_x, tr_y, t2hi, t2lo, 1, 1]
        #   rhs_ext  [6, HW] = [4fp_x, 4fp_y, 1, 1, f2hi, f2lo]
        # where t2 = -2|tr|^2 and f2 = -2|fp|^2, each split hi+lo in f32r so the
        # quadratic expansion of -2|fp - tr|^2 cancels exactly (all terms are
        # derived from the f32r-rounded coordinates). Each ext tile is written
        # by ONE DMA from flat partition-0 staging (wait-limit safety).
        eps_sb = singles.tile([128, 1], F32, tag="eps")
        nc.vector.memset(eps_sb, EPS)
        cm2 = singles.tile([1, 1], F32, tag="cm2")
        nc.vector.memset(cm2, -2.0)
        ext_q = singles.tile([6, HW], F32, tag="ext_q")
        g_all = singles.tile([128, 4], F32, tag="g_all")

        with tc.tile_pool(name="scratch", bufs=1) as scratch:
            c4 = scratch.tile([1, 1], F32, tag="c4")
            nc.vector.memset(c4, 4.0)
            c8 = scratch.tile([1, 1], F32, tag="c8")
            nc.vector.memset(c8, 0.125)

            gqk_sb = scratch.tile([1, 2 * D], F32, tag="gqk")
            nc.sync.dma_start(out=gqk_sb, in_=gqk.rearrange("d -> () d"))
            gflat = scratch.tile([1, D], F32, tag="gflat")
            nc.vector.tensor_mul(gflat, gqk_sb[:, 0:D], gqk_sb[:, D:2 * D])
            nc.vector.tensor_scalar_mul(out=gflat, in0=gflat, scalar1=c8)
            gperm = scratch.tile([1, D], F32, tag="gperm")
            nc.vector.tensor_copy(
                gperm.rearrange("x (p c) -> x p c", c=4),
                gflat.rearrange("x (c p) -> x p c", p=128),
            )

            fp_flat = scratch.tile([1, 2 * HW], F32, tag="fp_flat")
            nc.sync.dma_start(out=fp_flat, in_=fpT.rearrange("x q -> (x q)"))
            exq_flat = scratch.tile([1, 6 * HW], F32, tag="exq_flat")
            nc.vector.tensor_copy(exq_flat[:, 0:2 * HW], fp_flat)
            nc.vector.memset(exq_flat[:, 2 * HW:4 * HW], 1.0)
            sq_flat = scratch.tile([1, 2 * HW], F32, tag="fp_flat")
            nc.vector.tensor_mul(
                sq_flat,
                exq_flat[:, 0:2 * HW],
                exq_flat[:, 0:2 * HW],
            )
            nc.vector.tensor_scalar_mul(
                out=exq_flat[:, 0:2 * HW],
                in0=exq_flat[:, 0:2 * HW], scalar1=c4,
            )
            nfp = scratch.tile([1, HW], F32, tag="nfp")
            nc.vector.tensor_add(nfp, sq_flat[0:1, 0:HW], sq_flat[0:1, HW:2 * HW])
            nc.vector.tensor_scalar_mul(out=nfp, in0=nfp, scalar1=cm2)
            nc.vector.tensor_copy(exq_flat[:, 4 * HW:5 * HW], nfp)
            nc.vector.tensor_sub(
                exq_flat[:, 5 * HW:6 * HW], nfp,
                exq_flat[:, 4 * HW:5 * HW],
            )
            tc.strict_bb_all_engine_barrier()
            g_dram = dscr.tile([1, D], F32, tag="g_dram")
            nc.sync.dma_start(out=g_dram, in_=gperm)
            nc.sync.dma_start(out=g_all, in_=g_dram.rearrange("x (p c) -> x p c", c=4)[0])
            exq_dram = dscr.tile([1, 6 * HW], F32, tag="exq_dram")
            nc.sync.dma_start(out=exq_dram, in_=exq_flat)
            nc.sync.dma_start(out=ext_q, in_=exq_dram.rearrange("x (r q) -> x r q", r=6)[0])

        tc.strict_bb_all_engine_barrier()

        for t in range(t_per_core):
            # ---- per-t key-side ext rows, flat on partition 0, one DMA ----
            # tracks travel as raw f32 bytes inside the int8 actT tensor.
            trn_flat = small.tile([1, 2 * M], F32, tag="trn_flat")
            nc.sync.dma_start(
                out=trn_flat,
                in_=actT[t, :, TRK_C0:TRK_C0 + 4].bitcast(F32).rearrange(
                    "d one -> () (d one)"
                ),
            )
            trfr = small.tile([1, 2 * M], F32, tag="trfr")
            nc.vector.tensor_copy(trfr, trn_flat)
            trv = trfr.rearrange("x (k two) -> x k two", two=2)
            exk_flat = small.tile([1, 6 * M], F32, tag="exk_flat")
            nc.vector.tensor_copy(exk_flat[:, 0:M], trv[:, :, 0])
            nc.vector.tensor_copy(exk_flat[:, M:2 * M], trv[:, :, 1])
            nc.vector.memset(exk_flat[:, 4 * M:6 * M], 1.0)
            sqt = small.tile([1, 2 * M], F32, tag="sqt")
            nc.vector.tensor_mul(sqt, trfr, trfr)
            sqv = sqt.rearrange("x (k two) -> x k two", two=2)
            nrm = small.tile([1, M], F32, tag="nrm")
            nc.vector.tensor_add(nrm, sqv[:, :, 0], sqv[:, :, 1])
            nc.vector.tensor_scalar_mul(out=nrm, in0=nrm, scalar1=cm2)
            nc.vector.tensor_copy(exk_flat[:, 2 * M:3 * M], nrm)
            nc.vector.tensor_sub(
                exk_flat[:, 3 * M:4 * M], nrm, exk_flat[:, 2 * M:3 * M]
            )
            tick_dram = dscr.tile([1, 1], F32, tag="tick_dram")
            nc.sync.dma_start(out=tick_dram, in_=trn_flat[0:1, 0:1])
            exk_dram = dscr.tile([1, 6 * M], F32, tag="exk_dram")
            nc.sync.dma_start(out=exk_dram, in_=exk_flat)
            ext_k = small.tile([6, M], F32, tag="ext_k")
            nc.sync.dma_start(out=ext_k, in_=exk_dram.rearrange("x (r k) -> x r k", r=6)[0])

            # ---- load per-t activations (fpe/tpe int8 -> fp16; utt fp16) ----
            fpe_i8 = ins.tile([128, 4, HW], I8, tag="fpe_i8")
            nc.gpsimd.dma_start(out=fpe_i8, in_=actT[t, :, 0:HW].rearrange("(c p) q -> p c q", p=128))
            tpe_i8 = ins.tile([128, 4, M], I8, tag="tpe_i8")
            nc.gpsimd.dma_start(out=tpe_i8, in_=actT[t, :, HW:HW + M].rearrange("(c p) q -> p c q", p=128))
            utt_sb = ins.tile([128, 4, M], FP16, tag="utt")
            nc.gpsimd.dma_start(
                out=utt_sb,
                in_=actT[t, :, UTT_C0:UTT_C0 + 2 * M].bitcast(FP16).rearrange(
                    "(c p) q -> p c q", p=128
                ),
            )
            fpe_sb = ins.tile([128, 4, HW], FP16, tag="fpe")
            nc.vector.tensor_copy(fpe_sb, fpe_i8)
            tpe_sb = ins.tile([128, 4, M], FP16, tag="tpe")
            nc.vector.tensor_copy(tpe_sb, tpe_i8)

            # ---- projections + LN stats ----
            q_raw = work1.tile([128, 8, D], FP16, tag="q_raw")
            k_raw = work1.tile([128, 2, D], FP16, tag="k_raw")
            mv_all = work.tile([128, 10, 2], F32, tag="mv")
            for i in range(8):
                ps_q = pA.tile([128, D], F32, tag="pA")
                for c in range(4):
                    nc.tensor.matmul(
                        ps_q,
                        lhsT=fpe_sb[:, c, i * 128:(i + 1) * 128],
                        rhs=w_sb["wq"][:, c, :],
                        start=(c == 0), stop=(c == 3),
                    )
                nc.vector.tensor_copy(q_raw[:, i, :], ps_q)
                st = small.tile([128, 6], F32, tag="st")
                nc.vector.bn_stats(out=st, in_=q_raw[:, i, :])
                nc.vector.bn_aggr(out=mv_all[:, i, :], in_=st)
            for a in range(2):
                ps_k = pA.tile([128, D], F32, tag="pA")
                for c in range(4):
                    nc.tensor.matmul(
                        ps_k,
                        lhsT=tpe_sb[:, c, a * 128:(a + 1) * 128],
                        rhs=w_sb["wk"][:, c, :],
                        start=(c == 0), stop=(c == 3),
                    )
                nc.vector.tensor_copy(k_raw[:, a, :], ps_k)
                st = small.tile([128, 6], F32, tag="st")
                nc.vector.bn_stats(out=st, in_=k_raw[:, a, :])
                nc.vector.bn_aggr(out=mv_all[:, 8 + a, :], in_=st)

            # V projection straight into V-hat layout [k, 8 heads, 65]
            vhat = work1.tile([128, 2, H, 65], BF16, tag="vhat")
            nc.gpsimd.memset(vhat[:, :, :, 64:65], 1.0)
            for a in range(2):
                ps_v = pA.tile([128, D], F32, tag="pA")
                for c in range(4):
                    nc.tensor.matmul(
                        ps_v,
                        lhsT=utt_sb[:, c, a * 128:(a + 1) * 128],
                        rhs=w_sb["wv"][:, c, :],
                        start=(c == 0), stop=(c == 3),
                    )
                nc.vector.tensor_copy(
                    vhat[:, a, :, 0:64], ps_v.rearrange("p (h d) -> p h d", h=H)
                )

            # rstd = exp(-0.5 * ln(var + eps)) : stays in the exp table set
            rstd = work.tile([128, 10], F32, tag="rstd")
            nc.scalar.activation(out=rstd, in_=mv_all[:, :, 1], func=mybir.ActivationFunctionType.Ln, bias=eps_sb)
            nc.scalar.activation(out=rstd, in_=rstd, func=mybir.ActivationFunctionType.Exp, scale=-0.5)

            # ---- LN apply + transpose to [dk, q] ----
            q_ln = work1.tile([128, 8, D], FP16, tag="q_ln")
            for i in range(8):
                nc.vector.tensor_scalar(
                    out=q_ln[:, i, :], in0=q_raw[:, i, :],
                    scalar1=mv_all[:, i, 0:1], scalar2=rstd[:, i:i + 1],
                    op0=mybir.AluOpType.subtract, op1=mybir.AluOpType.mult,
                )
            k_ln = work1.tile([128, 2, D], FP16, tag="k_ln")
            for a in range(2):
                nc.vector.tensor_scalar(
                    out=k_ln[:, a, :], in0=k_raw[:, a, :],
                    scalar1=mv_all[:, 8 + a, 0:1], scalar2=rstd[:, 8 + a:9 + a],
                    op0=mybir.AluOpType.subtract, op1=mybir.AluOpType.mult,
                )

            qT = work1.tile([128, 4, HW], FP16, tag="qT")
            for c in range(4):
                for half in range(2):
                    ps_tr = pA.tile([128, D], FP16, tag="pT")
                    for j in range(4):
                        i = half * 4 + j
                        nc.tensor.transpose(
                            ps_tr[:, j * 128:(j + 1) * 128],
                            q_ln[:, i, c * 128:(c + 1) * 128], ident,
                        )
                    nc.vector.tensor_copy(qT[:, c, half * 512:(half + 1) * 512], ps_tr)
            kT = work1.tile([128, 4, M], FP16, tag="kT")
            for c in range(4):
                ps_tr = pA.tile([128, D], FP16, tag="pT")
                for a in range(2):
                    nc.tensor.transpose(
                        ps_tr[:, a * 128:(a + 1) * 128],
                        k_ln[:, a, c * 128:(c + 1) * 128], ident,
                    )
                # fold gamma_q*gamma_k/8 into the K side (per-partition here)
                nc.vector.tensor_scalar_mul(
                    out=kT[:, c, :], in0=ps_tr[:, 0:M], scalar1=g_all[:, c:c + 1]
                )

            # ---- scores + bias + exp, per (head, k-tile) ----
            exp_sb = {}
            for h in range(H):
                c, po = h // 2, (h % 2) * 64
                for a in range(2):
                    ps_s = pS.tile([128, 1024], F32, tag="pS")
                    for b in range(2):
                        sl = slice(b * 512, (b + 1) * 512)
                        nc.tensor.matmul(
                            ps_s[:, sl],
                            lhsT=kT[po:po + 64, c, a * 128:(a + 1) * 128],
                            rhs=qT[po:po + 64, c, sl],
                            start=True, stop=False,
                        )
                        nc.tensor.matmul(
                            ps_s[:, sl],
                            lhsT=ext_k[:, a * 128:(a + 1) * 128],
                            rhs=ext_q[:, sl],
                            start=False, stop=True,
                        )
                    es = exps.tile([128, HW], BF16, tag="exps")
                    nc.scalar.activation(out=es, in_=ps_s, func=mybir.ActivationFunctionType.Exp)
                    exp_sb[(h, a)] = es

            # ---- AV (U natural [q, 65] per head) + normalize ----
            u_norm = work1.tile([128, 8, D], FP16, tag="u_norm")
            for i in range(8):
                qsl = slice(i * 128, (i + 1) * 128)
                ps_u0 = pA.tile([128, 4, 65], F32, tag="pA")
                ps_u1 = pA.tile([128, 4, 65], F32, tag="pA")
                ps_u = [ps_u0, ps_u1]
                for h in range(H):
                    grp, slot = h // 4, h % 4
                    for a in range(2):
                        nc.tensor.matmul(
                            ps_u[grp][:, slot, :],
                            lhsT=exp_sb[(h, a)][:, qsl],
                            rhs=vhat[:, a, h, :],
                            start=(a == 0), stop=(a == 1),
                        )
                r8 = small.tile([128, 8], F32, tag="r8")
                for grp in range(2):
                    nc.vector.reciprocal(
                        out=r8[:, grp * 4:(grp + 1) * 4], in_=ps_u[grp][:, :, 64]
                    )
                for h in range(H):
                    grp, slot = h // 4, h % 4
                    nc.vector.tensor_scalar_mul(
                        out=u_norm[:, i, h * 64:(h + 1) * 64],
                        in0=ps_u[grp][:, slot, 0:64],
                        scalar1=r8[:, h:h + 1],
                    )

            # ---- transpose U, output projection, int8 quantize, store ----
            uT = work1.tile([128, 4, HW], FP16, tag="uT")
            for c in range(4):
                for half in range(2):
                    ps_tr = pA.tile([128, D], FP16, tag="pT")
                    for j in range(4):
                        i = half * 4 + j
                        nc.tensor.transpose(
                            ps_tr[:, j * 128:(j + 1) * 128],
                            u_norm[:, i, c * 128:(c + 1) * 128], ident,
                        )
                    nc.vector.tensor_copy(uT[:, c, half * 512:(half + 1) * 512], ps_tr)

            for i in range(8):
                ps_o = pA.tile([128, D], F32, tag="pA")
                for c in range(4):
                    nc.tensor.matmul(
                        ps_o,
                        lhsT=uT[:, c, i * 128:(i + 1) * 128],
                        rhs=w_sb["wo"][:, c, :],
                        start=(c == 0), stop=(c == 3),
                    )
                # per-row symmetric int8 quantization; |y| <= QSCALE + 0.5 < 127
                amax = small.tile([128, 1], F32, tag="amax")
                nc.vector.tensor_reduce(
                    out=amax, in_=ps_o, axis=mybir.AxisListType.X,
                    op=mybir.AluOpType.max, apply_absolute_value=True,
                )
                rinv = small.tile([128, 1], F32, tag="rinv")
                nc.vector.reciprocal(out=rinv, in_=amax)
                y = outs.tile([128, D], F32, tag="y")
                nc.vector.tensor_scalar(
                    out=y, in0=ps_o, scalar1=rinv, scalar2=QSCALE,
                    op0=mybir.AluOpType.mult, op1=mybir.AluOpType.mult,
                )
                # round-half-away-from-zero regardless of cast semantics
                sg = outs.tile([128, D], F32, tag="sg")
                nc.scalar.activation(out=sg, in_=y, func=mybir.ActivationFunctionType.Sign)
                nc.vector.tensor_scalar_mul(out=sg, in0=sg, scalar1=0.5)
                nc.vector.tensor_add(y, y, sg)
                o_i8 = outs.tile([128, OUT_W], I8, tag="o_i8")
                nc.vector.tensor_copy(o_i8[:, 0:D], y)
                sdq = small.tile([128, 1], F32, tag="sdq")
                nc.vector.tensor_scalar_mul(out=sdq, in0=amax, scalar1=1.0 / QSCALE)
                nc.vector.tensor_copy(o_i8[:, D:D + 4].bitcast(F32), sdq)
                nc.sync.dma_start(out=out[t, i * 128:(i + 1) * 128, :], in_=o_i8)

    nc.compile()
    return nc


_NC_CACHE = None  # (const_hash, nc)


_QBUF = {}


def _quant_rows(x):
    """Per-row symmetric int8 quantization over the last axis (fused, with
    reusable scratch to avoid 32MB allocation churn per call)."""
    buf = _QBUF.get(x.shape)
    if buf is None or buf.shape != x.shape:
        buf = _QBUF.setdefault(x.shape, np.empty_like(x))
    np.abs(x, out=buf)
    amax = np.maximum(buf.max(axis=-1), 1e-30)
    np.multiply(x, (QSCALE / amax)[..., None], out=buf)
    np.rint(buf, out=buf)
    return buf.astype(np.int8)


def kernel(**inputs) -> np.ndarray:
    global _NC_CACHE, LAST_RESULT
    import time as _time
    timing = bool(int(os.environ.get("KERNEL_TIMING", "0")))
    _t0 = _time.time()
    f32 = lambda x: np.ascontiguousarray(np.asarray(x, dtype=np.float32))
    fp16h = np.float16
    fpe = f32(inputs["feature_pos_embeddings"])      # [16, 1024, 512]
    tpe = f32(inputs["track_pos_embeddings"])        # [16, 256, 512]
    utt = f32(inputs["updated_track_tokens"])        # [16, 256, 512]
    tracks = f32(inputs["tracks"])                   # [16, 256, 2]
    fp = f32(inputs["feature_positions"])            # [1024, 2]

    # One packed int8 tensor per timestep: [D, HW | M | 2M | 4].
    # fpe/tpe are int8 (quantization scales cancel in the on-device QK
    # LayerNorm); utt travels as raw fp16 bytes; tracks as raw f32 bytes.
    T = N_CORES * T_PER_CORE
    actT = np.empty((T, D, ACT_W), dtype=np.int8)
    actT[:, :, 0:HW] = _quant_rows(fpe).transpose(0, 2, 1)
    actT[:, :, HW:HW + M] = _quant_rows(tpe).transpose(0, 2, 1)
    actT[:, :, UTT_C0:UTT_C0 + 2 * M] = np.ascontiguousarray(
        utt.astype(fp16h).transpose(0, 2, 1)
    ).view(np.int8)
    actT[:, :, TRK_C0:TRK_C0 + 4] = tracks.reshape(T, 2 * M, 1).view(np.int8)

    consts = {
        "wqT": np.ascontiguousarray(f32(inputs["W_q"]).T.astype(fp16h)),
        "wkT": np.ascontiguousarray(f32(inputs["W_k"]).T.astype(fp16h)),
        "wvT": np.ascontiguousarray(f32(inputs["W_v"]).T.astype(fp16h)),
        "woT": np.ascontiguousarray(f32(inputs["W_out"]).T.astype(fp16h)),
        "gqk": np.ascontiguousarray(
            np.concatenate([f32(inputs["q_gamma"]), f32(inputs["k_gamma"])])
        ),
        "fpT": np.ascontiguousarray(fp.T),
    }
    h = hashlib.md5()
    for k in sorted(consts):
        h.update(k.encode())
        h.update(consts[k].tobytes())
    chash = h.hexdigest()

    _t1 = _time.time()
    if _NC_CACHE is None or _NC_CACHE[0] != chash:
        _NC_CACHE = (chash, _build_bass(consts))
    nc = _NC_CACHE[1]
    _t2 = _time.time()

    in_maps = []
    for core in range(N_CORES):
        t0 = core * T_PER_CORE
        in_maps.append({"actT": actT[t0:t0 + T_PER_CORE]})

    want_trace = bool(int(os.environ.get("KERNEL_TRACE", "0")))
    try:
        res = bass_utils.run_bass_kernel_spmd(
            nc, in_maps, core_ids=list(range(N_CORES)), trace=want_trace,
        )
    except ModuleNotFoundError:
        res = bass_utils.run_bass_kernel_spmd(
            nc, in_maps, core_ids=list(range(N_CORES)), trace=False,
        )
    _t3 = _time.time()
    LAST_RESULT = res
    result = np.empty((T, HW, D), np.float32)
    for core, r in enumerate(res.results):
        raw = r["out"]  # [T_PER_CORE, HW, OUT_W] int8
        t0 = core * T_PER_CORE
        scales = np.ascontiguousarray(raw[:, :, D:D + 4]).view(np.float32)
        np.multiply(raw[:, :, 0:D], scales, out=result[t0:t0 + T_PER_CORE])
    if timing:
        print(
            f"[ktime] prep={_t1 - _t0:.3f}s build={_t2 - _t1:.3f}s "
            f"spmd={_t3 - _t2:.3f}s post={_time.time() - _t3:.3f}s"
        )
    return result


# revision 25
# speedup vs baseline: 1.1507x; 1.1507x over previous
"""AttentionalSplatting TRN2 kernel.

Sharding: data-parallel over T (16 timesteps) across 8 cores, issued as two
pipelined spmd calls of 1 timestep/core each: the second call starts
PIPE_STAGGER after the first, so its upload overlaps the first call's
readback on the full-duplex axon link (~0.15s saved per kernel() call).
Weights are baked into the NEFF as constants (loaded to HBM once at
model-load; they never travel per call). Host does layout permutations,
dtype casts and int8 quantization only; all arithmetic runs on device.

Per-call traffic is the optimization target on this link (~40 MB/s H2D,
~25 MB/s D2H, ~80 ms RTT per op):
  - ONE packed int8 input tensor per core [T_PER_CORE, D, 1796]:
    columns [fpeT_i8 | tpeT_i8 | uttT-as-raw-fp16-bytes |
    tracks-as-raw-f32-bytes]. fpe/tpe rows are quantized per token row
    with NO shipped scale: QK-LayerNorm is scale-invariant per row, so
    the quantization scale cancels exactly on device. utt (V path) has
    no LN to absorb a scale and int8 was too lossy there, so it travels
    as fp16 raw bytes recovered with AP.bitcast. tracks need exact f32
    for the bias-cancellation trick and ride along the same way.
  - Output is int8 [T_PER_CORE, HW, 516]: 512 quantized values per row
    plus that row's f32 dequant scale bit-packed in the last 4 bytes.
    Host dequant is one fused np.multiply.
  - run_bass_via_pjrt rebuilds its jit wrapper per call, defeating jax's
    compile caches; two content-keyed memos below (neuronx_cc hook and
    backend_compile_and_load) recover the ~0.6 s/call that was spent
    re-verifying the identical BIR and re-shipping the identical
    executable. Data transfer + device execution still happen per call.

Per-timestep device pipeline (fp16 matmuls, fp32 softmax/LN statistics;
exp/V tiles are bf16 because fp16's 6e-8 floor underflows to a zero
softmax denominator for far-from-every-track queries — row-max scores
reach -40 on real inputs, and bf16's 1e-38 range absorbs that):
  Q = fpe @ WqT   (natural [q, dk] layout, PSUM)    -> LN stats -> apply -> fp16
  K = tpe @ WkT   likewise; V = utt @ WvT -> V-hat [k, 8, 65] with ones col
  Qln/Kln PE-transposed to [dk, q]; gamma_q*gamma_k/8 folded into K side.
  scoresT[k,q] per head = Kh^T.T @ Qh^T  (+ spatial bias via a rank-6 f32r
  matmul on appended position rows: -2*d2 = 4 tr.fp - 2|tr|^2 - 2|fp|^2)
  exp on ACT (no max subtraction needed: bias <= 0, |QK/8| small)
  U_h[q, 65] = expS^T.T @ Vhat_h  (col 64 = softmax denom) -> recip -> scale
  out = U @ WoT via PE transpose of U, per-row int8 quantization, DMA out.
"""

import hashlib
import os
from contextlib import ExitStack

import numpy as np

import concourse.bass as bass
import concourse.mybir as mybir
import concourse.tile as tile
from concourse import bacc, bass2jax, bass_utils
from concourse.masks import make_identity

# run_bass_via_pjrt rebuilds its jitted callable every call, so jax's
# compile caches miss and concourse's neuronx_cc hook re-runs the full
# BIR verify/optimise + DVE-table pipeline (~0.6 s of client CPU) on
# byte-identical input each call. The hook is a pure function of the
# serialized HLO (the NEFF repack is explicitly made deterministic via
# _reset_tarinfo/make_deterministic_neff_header), so memoize it by
# content hash — same role as the NEFF compile cache in the bench
# containers. Execution itself still runs on hardware every call.
_NEFF_MEMO = {}
_ORIG_NEURONX_CC_HOOK = bass2jax.neuronx_cc_hook


def _memo_neuronx_cc_hook(code, code_format, platform_version, file_prefix):
    try:
        key = (
            hashlib.sha256(bytes(code)).digest(),
            bytes(code_format),
            str(platform_version),
        )
    except Exception:
        return _ORIG_NEURONX_CC_HOOK(code, code_format, platform_version, file_prefix)
    hit = _NEFF_MEMO.get(key)
    if hit is None:
        hit = _ORIG_NEURONX_CC_HOOK(code, code_format, platform_version, file_prefix)
        _NEFF_MEMO[key] = hit
    return hit


bass2jax.neuronx_cc_hook = _memo_neuronx_cc_hook

# Same story one level up: jax's own executable cache keys miss for the
# per-call-rebuilt jit wrapper, so every call re-runs backend.compile
# (client-side NEFF wrap + shipping the executable to the axon terminal,
# ~0.6 s). The compile is deterministic in (module bytecode, compile
# options, devices) — memoize the loaded executable on that key, exactly
# the hit-path jax's _cached_compilation intends. Fall through to the
# original on any surprise.
_EXE_MEMO = {}


def _install_compile_memo():
    import jax._src.compiler as _jc
    from jax._src.interpreters import mlir as _jmlir

    if getattr(_jc.backend_compile_and_load, "_kernel_memo", False):
        return
    _orig = _jc.backend_compile_and_load

    def _memo(backend, module, executable_devices, options, host_callbacks):
        try:
            if host_callbacks:
                return _orig(
                    backend, module, executable_devices, options, host_callbacks
                )
            key = (
                hashlib.sha256(_jmlir.module_to_bytecode(module)).digest(),
                options.SerializeAsString(),
                tuple(d.id for d in executable_devices),
            )
        except Exception:
            return _orig(backend, module, executable_devices, options, host_callbacks)
        hit = _EXE_MEMO.get(key)
        if hit is None:
            hit = _orig(backend, module, executable_devices, options, host_callbacks)
            _EXE_MEMO[key] = hit
        return hit

    _memo._kernel_memo = True
    _jc.backend_compile_and_load = _memo


_install_compile_memo()

F32 = mybir.dt.float32
F32R = mybir.dt.float32r
BF16 = mybir.dt.bfloat16
FP16 = mybir.dt.float16
I8 = mybir.dt.int8

T_PER_CORE = 1   # timesteps per core per spmd call; 2 pipelined calls cover T=16
N_CORES = 8
PIPE_STAGGER = 0.2  # s; lets call B's upload overlap call A's readback (duplex link)
HW = 1024  # queries
M = 256    # tracks/keys
D = 512    # d_model = d_k
H = 8
HD = 64
EPS = 1e-6
ACT_W = HW + M + 2 * M + 4  # packed: [fpeT i8 | tpeT i8 | uttT fp16-bytes | trk]
UTT_C0 = HW + M             # utt fp16 region start (as int8 columns)
TRK_C0 = HW + 3 * M         # tracks f32 bytes region start
OUT_W = D + 4           # int8 values + bit-packed f32 row scale
QSCALE = 126.5          # int8 target amplitude (rounding can add 0.5)

LAST_RESULT = None


def _build_bass(consts, t_per_core=T_PER_CORE):
    nc = bacc.Bacc("TRN2", target_bir_lowering=False)

    actT = nc.dram_tensor("actT", [t_per_core, D, ACT_W], I8, kind="ExternalInput").ap()
    out = nc.dram_tensor("out", [t_per_core, HW, OUT_W], I8, kind="ExternalOutput").ap()

    # Weight-like tensors baked into the NEFF: loaded to HBM at model load,
    # zero per-call transfer cost.
    wqT = nc.inline_tensor(consts["wqT"], name="wqT").ap()
    wkT = nc.inline_tensor(consts["wkT"], name="wkT").ap()
    wvT = nc.inline_tensor(consts["wvT"], name="wvT").ap()
    woT = nc.inline_tensor(consts["woT"], name="woT").ap()
    gqk = nc.inline_tensor(consts["gqk"], name="gqk").ap()
    fpT = nc.inline_tensor(consts["fpT"], name="fpT").ap()

    with tile.TileContext(nc) as tc, ExitStack() as ctx:
        singles = ctx.enter_context(tc.tile_pool(name="singles", bufs=1))
        ins = ctx.enter_context(tc.tile_pool(name="ins", bufs=1))
        work = ctx.enter_context(tc.tile_pool(name="work", bufs=2))
        work1 = ctx.enter_context(tc.tile_pool(name="work1", bufs=1))
        small = ctx.enter_context(tc.tile_pool(name="small", bufs=2))
        exps = ctx.enter_context(tc.tile_pool(name="exps", bufs=16))
        outs = ctx.enter_context(tc.tile_pool(name="outs", bufs=2))
        pA = ctx.enter_context(tc.tile_pool(name="pA", bufs=2, space="PSUM"))
        pS = ctx.enter_context(tc.tile_pool(name="pS", bufs=2, space="PSUM"))
        dscr = ctx.enter_context(tc.tile_pool(name="dscr", bufs=2, space="DRAM"))

        # ---- one-time constants ----
        ident = singles.tile([128, 128], FP16)
        make_identity(nc, ident)

        w_sb = {}
        for name, ap in (("wq", wqT), ("wk", wkT), ("wv", wvT), ("wo", woT)):
            wt = singles.tile([128, 4, D], FP16, tag=name)
            nc.gpsimd.dma_start(out=wt, in_=ap.rearrange("(c p) n -> p c n", p=128))
            w_sb[name] = wt

        # ext rows (rank-6 bias matmul):
        #   lhsT_ext [6, M]  = [tr_x, tr_y, t2hi, t2lo, 1, 1]
        #   rhs_ext  [6, HW] = [4fp_x, 4fp_y, 1, 1, f2hi, f2lo]
        # where t2 = -2|tr|^2 and f2 = -2|fp|^2, each split hi+lo in f32r so the
        # quadratic expansion of -2|fp - tr|^2 cancels exactly (all terms are
        # derived from the f32r-rounded coordinates). Each ext tile is written
        # by ONE DMA from flat partition-0 staging (wait-limit safety).
        eps_sb = singles.tile([128, 1], F32, tag="eps")
        nc.vector.memset(eps_sb, EPS)
        cm2 = singles.tile([1, 1], F32, tag="cm2")
        nc.vector.memset(cm2, -2.0)
        ext_q = singles.tile([6, HW], F32, tag="ext_q")
        g_all = singles.tile([128, 4], F32, tag="g_all")

        with tc.tile_pool(name="scratch", bufs=1) as scratch:
            c4 = scratch.tile([1, 1], F32, tag="c4")
            nc.vector.memset(c4, 4.0)
            c8 = scratch.tile([1, 1], F32, tag="c8")
            nc.vector.memset(c8, 0.125)

            gqk_sb = scratch.tile([1, 2 * D], F32, tag="gqk")
            nc.sync.dma_start(out=gqk_sb, in_=gqk.rearrange("d -> () d"))
            gflat = scratch.tile([1, D], F32, tag="gflat")
            nc.vector.tensor_mul(gflat, gqk_sb[:, 0:D], gqk_sb[:, D:2 * D])
            nc.vector.tensor_scalar_mul(out=gflat, in0=gflat, scalar1=c8)
            gperm = scratch.tile([1, D], F32, tag="gperm")
            nc.vector.tensor_copy(
                gperm.rearrange("x (p c) -> x p c", c=4),
                gflat.rearrange("x (c p) -> x p c", p=128),
            )

            fp_flat = scratch.tile([1, 2 * HW], F32, tag="fp_flat")
            nc.sync.dma_start(out=fp_flat, in_=fpT.rearrange("x q -> (x q)"))
            exq_flat = scratch.tile([1, 6 * HW], F32, tag="exq_flat")
            nc.vector.tensor_copy(exq_flat[:, 0:2 * HW], fp_flat)
            nc.vector.memset(exq_flat[:, 2 * HW:4 * HW], 1.0)
            sq_flat = scratch.tile([1, 2 * HW], F32, tag="fp_flat")
            nc.vector.tensor_mul(
                sq_flat,
                exq_flat[:, 0:2 * HW],
                exq_flat[:, 0:2 * HW],
            )
            nc.vector.tensor_scalar_mul(
                out=exq_flat[:, 0:2 * HW],
                in0=exq_flat[:, 0:2 * HW], scalar1=c4,
            )
            nfp = scratch.tile([1, HW], F32, tag="nfp")
            nc.vector.tensor_add(nfp, sq_flat[0:1, 0:HW], sq_flat[0:1, HW:2 * HW])
            nc.vector.tensor_scalar_mul(out=nfp, in0=nfp, scalar1=cm2)
            nc.vector.tensor_copy(exq_flat[:, 4 * HW:5 * HW], nfp)
            nc.vector.tensor_sub(
                exq_flat[:, 5 * HW:6 * HW], nfp,
                exq_flat[:, 4 * HW:5 * HW],
            )
            tc.strict_bb_all_engine_barrier()
            g_dram = dscr.tile([1, D], F32, tag="g_dram")
            nc.sync.dma_start(out=g_dram, in_=gperm)
            nc.sync.dma_start(out=g_all, in_=g_dram.rearrange("x (p c) -> x p c", c=4)[0])
            exq_dram = dscr.tile([1, 6 * HW], F32, tag="exq_dram")
            nc.sync.dma_start(out=exq_dram, in_=exq_flat)
            nc.sync.dma_start(out=ext_q, in_=exq_dram.rearrange("x (r q) -> x r q", r=6)[0])

        tc.strict_bb_all_engine_barrier()

        for t in range(t_per_core):
            # ---- per-t key-side ext rows, flat on partition 0, one DMA ----
            # tracks travel as raw f32 bytes inside the int8 actT tensor.
            trn_flat = small.tile([1, 2 * M], F32, tag="trn_flat")
            nc.sync.dma_start(
                out=trn_flat,
                in_=actT[t, :, TRK_C0:TRK_C0 + 4].bitcast(F32).rearrange(
                    "d one -> () (d one)"
                ),
            )
            trfr = small.tile([1, 2 * M], F32, tag="trfr")
            nc.vector.tensor_copy(trfr, trn_flat)
            trv = trfr.rearrange("x (k two) -> x k two", two=2)
            exk_flat = small.tile([1, 6 * M], F32, tag="exk_flat")
            nc.vector.tensor_copy(exk_flat[:, 0:M], trv[:, :, 0])
            nc.vector.tensor_copy(exk_flat[:, M:2 * M], trv[:, :, 1])
            nc.vector.memset(exk_flat[:, 4 * M:6 * M], 1.0)
            sqt = small.tile([1, 2 * M], F32, tag="sqt")
            nc.vector.tensor_mul(sqt, trfr, trfr)
            sqv = sqt.rearrange("x (k two) -> x k two", two=2)
            nrm = small.tile([1, M], F32, tag="nrm")
            nc.vector.tensor_add(nrm, sqv[:, :, 0], sqv[:, :, 1])
            nc.vector.tensor_scalar_mul(out=nrm, in0=nrm, scalar1=cm2)
            nc.vector.tensor_copy(exk_flat[:, 2 * M:3 * M], nrm)
            nc.vector.tensor_sub(
                exk_flat[:, 3 * M:4 * M], nrm, exk_flat[:, 2 * M:3 * M]
            )
            tick_dram = dscr.tile([1, 1], F32, tag="tick_dram")
            nc.sync.dma_start(out=tick_dram, in_=trn_flat[0:1, 0:1])
            exk_dram = dscr.tile([1, 6 * M], F32, tag="exk_dram")
            nc.sync.dma_start(out=exk_dram, in_=exk_flat)
            ext_k = small.tile([6, M], F32, tag="ext_k")
            nc.sync.dma_start(out=ext_k, in_=exk_dram.rearrange("x (r k) -> x r k", r=6)[0])

            # ---- load per-t activations (fpe/tpe int8 -> fp16; utt fp16) ----
            fpe_i8 = ins.tile([128, 4, HW], I8, tag="fpe_i8")
            nc.gpsimd.dma_start(out=fpe_i8, in_=actT[t, :, 0:HW].rearrange("(c p) q -> p c q", p=128))
            tpe_i8 = ins.tile([128, 4, M], I8, tag="tpe_i8")
            nc.gpsimd.dma_start(out=tpe_i8, in_=actT[t, :, HW:HW + M].rearrange("(c p) q -> p c q", p=128))
            utt_sb = ins.tile([128, 4, M], FP16, tag="utt")
            nc.gpsimd.dma_start(
                out=utt_sb,
                in_=actT[t, :, UTT_C0:UTT_C0 + 2 * M].bitcast(FP16).rearrange(
                    "(c p) q -> p c q", p=128
                ),
            )
            fpe_sb = ins.tile([128, 4, HW], FP16, tag="fpe")
            nc.vector.tensor_copy(fpe_sb, fpe_i8)
            tpe_sb = ins.tile([128, 4, M], FP16, tag="tpe")
            nc.vector.tensor_copy(tpe_sb, tpe_i8)

            # ---- projections + LN stats ----
            q_raw = work1.tile([128, 8, D], FP16, tag="q_raw")
            k_raw = work1.tile([128, 2, D], FP16, tag="k_raw")
            mv_all = work.tile([128, 10, 2], F32, tag="mv")
            for i in range(8):
                ps_q = pA.tile([128, D], F32, tag="pA")
                for c in range(4):
                    nc.tensor.matmul(
                        ps_q,
                        lhsT=fpe_sb[:, c, i * 128:(i + 1) * 128],
                        rhs=w_sb["wq"][:, c, :],
                        start=(c == 0), stop=(c == 3),
                    )
                nc.vector.tensor_copy(q_raw[:, i, :], ps_q)
                st = small.tile([128, 6], F32, tag="st")
                nc.vector.bn_stats(out=st, in_=q_raw[:, i, :])
                nc.vector.bn_aggr(out=mv_all[:, i, :], in_=st)
            for a in range(2):
                ps_k = pA.tile([128, D], F32, tag="pA")
                for c in range(4):
                    nc.tensor.matmul(
                        ps_k,
                        lhsT=tpe_sb[:, c, a * 128:(a + 1) * 128],
                        rhs=w_sb["wk"][:, c, :],
                        start=(c == 0), stop=(c == 3),
                    )
                nc.vector.tensor_copy(k_raw[:, a, :], ps_k)
                st = small.tile([128, 6], F32, tag="st")
                nc.vector.bn_stats(out=st, in_=k_raw[:, a, :])
                nc.vector.bn_aggr(out=mv_all[:, 8 + a, :], in_=st)

            # V projection straight into V-hat layout [k, 8 heads, 65]
            vhat = work1.tile([128, 2, H, 65], BF16, tag="vhat")
            nc.gpsimd.memset(vhat[:, :, :, 64:65], 1.0)
            for a in range(2):
                ps_v = pA.tile([128, D], F32, tag="pA")
                for c in range(4):
                    nc.tensor.matmul(
                        ps_v,
                        lhsT=utt_sb[:, c, a * 128:(a + 1) * 128],
                        rhs=w_sb["wv"][:, c, :],
                        start=(c == 0), stop=(c == 3),
                    )
                nc.vector.tensor_copy(
                    vhat[:, a, :, 0:64], ps_v.rearrange("p (h d) -> p h d", h=H)
                )

            # rstd = exp(-0.5 * ln(var + eps)) : stays in the exp table set
            rstd = work.tile([128, 10], F32, tag="rstd")
            nc.scalar.activation(out=rstd, in_=mv_all[:, :, 1], func=mybir.ActivationFunctionType.Ln, bias=eps_sb)
            nc.scalar.activation(out=rstd, in_=rstd, func=mybir.ActivationFunctionType.Exp, scale=-0.5)

            # ---- LN apply + transpose to [dk, q] ----
            q_ln = work1.tile([128, 8, D], FP16, tag="q_ln")
            for i in range(8):
                nc.vector.tensor_scalar(
                    out=q_ln[:, i, :], in0=q_raw[:, i, :],
                    scalar1=mv_all[:, i, 0:1], scalar2=rstd[:, i:i + 1],
                    op0=mybir.AluOpType.subtract, op1=mybir.AluOpType.mult,
                )
            k_ln = work1.tile([128, 2, D], FP16, tag="k_ln")
            for a in range(2):
                nc.vector.tensor_scalar(
                    out=k_ln[:, a, :], in0=k_raw[:, a, :],
                    scalar1=mv_all[:, 8 + a, 0:1], scalar2=rstd[:, 8 + a:9 + a],
                    op0=mybir.AluOpType.subtract, op1=mybir.AluOpType.mult,
                )

            qT = work1.tile([128, 4, HW], FP16, tag="qT")
            for c in range(4):
                for half in range(2):
                    ps_tr = pA.tile([128, D], FP16, tag="pT")
                    for j in range(4):
                        i = half * 4 + j
                        nc.tensor.transpose(
                            ps_tr[:, j * 128:(j + 1) * 128],
                            q_ln[:, i, c * 128:(c + 1) * 128], ident,
                        )
                    nc.vector.tensor_copy(qT[:, c, half * 512:(half + 1) * 512], ps_tr)
            kT = work1.tile([128, 4, M], FP16, tag="kT")
            for c in range(4):
                ps_tr = pA.tile([128, D], FP16, tag="pT")
                for a in range(2):
                    nc.tensor.transpose(
                        ps_tr[:, a * 128:(a + 1) * 128],
                        k_ln[:, a, c * 128:(c + 1) * 128], ident,
                    )
                # fold gamma_q*gamma_k/8 into the K side (per-partition here)
                nc.vector.tensor_scalar_mul(
                    out=kT[:, c, :], in0=ps_tr[:, 0:M], scalar1=g_all[:, c:c + 1]
                )

            # ---- scores + bias + exp, per (head, k-tile) ----
            exp_sb = {}
            for h in range(H):
                c, po = h // 2, (h % 2) * 64
                for a in range(2):
                    ps_s = pS.tile([128, 1024], F32, tag="pS")
                    for b in range(2):
                        sl = slice(b * 512, (b + 1) * 512)
                        nc.tensor.matmul(
                            ps_s[:, sl],
                            lhsT=kT[po:po + 64, c, a * 128:(a + 1) * 128],
                            rhs=qT[po:po + 64, c, sl],
                            start=True, stop=False,
                        )
                        nc.tensor.matmul(
                            ps_s[:, sl],
                            lhsT=ext_k[:, a * 128:(a + 1) * 128],
                            rhs=ext_q[:, sl],
                            start=False, stop=True,
                        )
                    es = exps.tile([128, HW], BF16, tag="exps")
                    nc.scalar.activation(out=es, in_=ps_s, func=mybir.ActivationFunctionType.Exp)
                    exp_sb[(h, a)] = es

            # ---- AV (U natural [q, 65] per head) + normalize ----
            u_norm = work1.tile([128, 8, D], FP16, tag="u_norm")
            for i in range(8):
                qsl = slice(i * 128, (i + 1) * 128)
                ps_u0 = pA.tile([128, 4, 65], F32, tag="pA")
                ps_u1 = pA.tile([128, 4, 65], F32, tag="pA")
                ps_u = [ps_u0, ps_u1]
                for h in range(H):
                    grp, slot = h // 4, h % 4
                    for a in range(2):
                        nc.tensor.matmul(
                            ps_u[grp][:, slot, :],
                            lhsT=exp_sb[(h, a)][:, qsl],
                            rhs=vhat[:, a, h, :],
                            start=(a == 0), stop=(a == 1),
                        )
                r8 = small.tile([128, 8], F32, tag="r8")
                for grp in range(2):
                    nc.vector.reciprocal(
                        out=r8[:, grp * 4:(grp + 1) * 4], in_=ps_u[grp][:, :, 64]
                    )
                for h in range(H):
                    grp, slot = h // 4, h % 4
                    nc.vector.tensor_scalar_mul(
                        out=u_norm[:, i, h * 64:(h + 1) * 64],
                        in0=ps_u[grp][:, slot, 0:64],
                        scalar1=r8[:, h:h + 1],
                    )

            # ---- transpose U, output projection, int8 quantize, store ----
            uT = work1.tile([128, 4, HW], FP16, tag="uT")
            for c in range(4):
                for half in range(2):
                    ps_tr = pA.tile([128, D], FP16, tag="pT")
                    for j in range(4):
                        i = half * 4 + j
                        nc.tensor.transpose(
                            ps_tr[:, j * 128:(j + 1) * 128],
                            u_norm[:, i, c * 128:(c + 1) * 128], ident,
                        )
                    nc.vector.tensor_copy(uT[:, c, half * 512:(half + 1) * 512], ps_tr)

            for i in range(8):
                ps_o = pA.tile([128, D], F32, tag="pA")
                for c in range(4):
                    nc.tensor.matmul(
                        ps_o,
                        lhsT=uT[:, c, i * 128:(i + 1) * 128],
                        rhs=w_sb["wo"][:, c, :],
                        start=(c == 0), stop=(c == 3),
                    )
                # per-row symmetric int8 quantization; |y| <= QSCALE + 0.5 < 127
                amax = small.tile([128, 1], F32, tag="amax")
                nc.vector.tensor_reduce(
                    out=amax, in_=ps_o, axis=mybir.AxisListType.X,
                    op=mybir.AluOpType.max, apply_absolute_value=True,
                )
                rinv = small.tile([128, 1], F32, tag="rinv")
                nc.vector.reciprocal(out=rinv, in_=amax)
                y = outs.tile([128, D], F32, tag="y")
                nc.vector.tensor_scalar(
                    out=y, in0=ps_o, scalar1=rinv, scalar2=QSCALE,
                    op0=mybir.AluOpType.mult, op1=mybir.AluOpType.mult,
                )
                # round-half-away-from-zero regardless of cast semantics
                sg = outs.tile([128, D], F32, tag="sg")
                nc.scalar.activation(out=sg, in_=y, func=mybir.ActivationFunctionType.Sign)
                nc.vector.tensor_scalar_mul(out=sg, in0=sg, scalar1=0.5)
                nc.vector.tensor_add(y, y, sg)
                o_i8 = outs.tile([128, OUT_W], I8, tag="o_i8")
                nc.vector.tensor_copy(o_i8[:, 0:D], y)
                sdq = small.tile([128, 1], F32, tag="sdq")
                nc.vector.tensor_scalar_mul(out=sdq, in0=amax, scalar1=1.0 / QSCALE)
                nc.vector.tensor_copy(o_i8[:, D:D + 4].bitcast(F32), sdq)
                nc.sync.dma_start(out=out[t, i * 128:(i + 1) * 128, :], in_=o_i8)

    nc.compile()
    return nc


_NC_CACHE = None  # (const_hash, nc)
_PIPE_WARM = False


_QBUF = {}


def _quant_rows(x):
    """Per-row symmetric int8 quantization over the last axis (fused, with
    reusable scratch to avoid 32MB allocation churn per call)."""
    buf = _QBUF.get(x.shape)
    if buf is None or buf.shape != x.shape:
        buf = _QBUF.setdefault(x.shape, np.empty_like(x))
    np.abs(x, out=buf)
    amax = np.maximum(buf.max(axis=-1), 1e-30)
    np.multiply(x, (QSCALE / amax)[..., None], out=buf)
    np.rint(buf, out=buf)
    return buf.astype(np.int8)


def kernel(**inputs) -> np.ndarray:
    global _NC_CACHE, LAST_RESULT
    import time as _time
    timing = bool(int(os.environ.get("KERNEL_TIMING", "0")))
    _t0 = _time.time()
    f32 = lambda x: np.ascontiguousarray(np.asarray(x, dtype=np.float32))
    fp16h = np.float16
    fpe = f32(inputs["feature_pos_embeddings"])      # [16, 1024, 512]
    tpe = f32(inputs["track_pos_embeddings"])        # [16, 256, 512]
    utt = f32(inputs["updated_track_tokens"])        # [16, 256, 512]
    tracks = f32(inputs["tracks"])                   # [16, 256, 2]
    fp = f32(inputs["feature_positions"])            # [1024, 2]

    # One packed int8 tensor per timestep: [D, HW | M | 2M | 4].
    # fpe/tpe are int8 (quantization scales cancel in the on-device QK
    # LayerNorm); utt travels as raw fp16 bytes; tracks as raw f32 bytes.
    T = 2 * N_CORES  # total timesteps in the problem (16)
    actT = np.empty((T, D, ACT_W), dtype=np.int8)
    actT[:, :, 0:HW] = _quant_rows(fpe).transpose(0, 2, 1)
    actT[:, :, HW:HW + M] = _quant_rows(tpe).transpose(0, 2, 1)
    actT[:, :, UTT_C0:UTT_C0 + 2 * M] = np.ascontiguousarray(
        utt.astype(fp16h).transpose(0, 2, 1)
    ).view(np.int8)
    actT[:, :, TRK_C0:TRK_C0 + 4] = tracks.reshape(T, 2 * M, 1).view(np.int8)

    consts = {
        "wqT": np.ascontiguousarray(f32(inputs["W_q"]).T.astype(fp16h)),
        "wkT": np.ascontiguousarray(f32(inputs["W_k"]).T.astype(fp16h)),
        "wvT": np.ascontiguousarray(f32(inputs["W_v"]).T.astype(fp16h)),
        "woT": np.ascontiguousarray(f32(inputs["W_out"]).T.astype(fp16h)),
        "gqk": np.ascontiguousarray(
            np.concatenate([f32(inputs["q_gamma"]), f32(inputs["k_gamma"])])
        ),
        "fpT": np.ascontiguousarray(fp.T),
    }
    h = hashlib.md5()
    for k in sorted(consts):
        h.update(k.encode())
        h.update(consts[k].tobytes())
    chash = h.hexdigest()

    _t1 = _time.time()
    if _NC_CACHE is None or _NC_CACHE[0] != chash:
        _NC_CACHE = (chash, _build_bass(consts))
    nc = _NC_CACHE[1]
    _t2 = _time.time()

    want_trace = bool(int(os.environ.get("KERNEL_TRACE", "0")))
    result = np.empty((T, HW, D), np.float32)

    def _run_half(h0):
        in_maps = [{"actT": actT[h0 + c:h0 + c + 1]} for c in range(N_CORES)]
        try:
            res = bass_utils.run_bass_kernel_spmd(
                nc, in_maps, core_ids=list(range(N_CORES)), trace=want_trace,
            )
        except ModuleNotFoundError:
            res = bass_utils.run_bass_kernel_spmd(
                nc, in_maps, core_ids=list(range(N_CORES)), trace=False,
            )
        for c, r in enumerate(res.results):
            raw = r["out"]  # [1, HW, OUT_W] int8
            scales = np.ascontiguousarray(raw[:, :, D:D + 4]).view(np.float32)
            np.multiply(raw[:, :, 0:D], scales, out=result[h0 + c:h0 + c + 1])
        return res

    global _PIPE_WARM
    if not _PIPE_WARM:
        # First call in the process: run halves sequentially so the compile
        # path (memo miss) isn't raced by two threads.
        _run_half(0)
        LAST_RESULT = _run_half(N_CORES)
        _PIPE_WARM = True
    else:
        import threading
        boxes = [None, None]

        def _worker(idx, h0, delay):
            try:
                if delay:
                    _time.sleep(delay)
                boxes[idx] = _run_half(h0)
            except BaseException as e:  # surfaced in the main thread below
                boxes[idx] = e

        ta = threading.Thread(target=_worker, args=(0, 0, 0.0))
        tb = threading.Thread(target=_worker, args=(1, N_CORES, PIPE_STAGGER))
        ta.start(); tb.start(); ta.join(); tb.join()
        for b in boxes:
            if isinstance(b, BaseException):
                raise b
        LAST_RESULT = boxes[1]
    _t3 = _time.time()
    if timing:
        print(
            f"[ktime] prep={_t1 - _t0:.3f}s build={_t2 - _t1:.3f}s "
            f"spmd+post={_t3 - _t2:.3f}s"
        )
    return result
